# revision 17
# baseline (speedup 1.0000x reference)
"""MoE routing layer on 8 Trainium2 NeuronCores (data-parallel over batch).

Per core (4 samples):
  routing MLP -> cosine sim vs embeddings -> softmax weights wf[4,10]
  w_eff[b] = sum_n wf[b,n] * conv_w[n]  (conv is linear in weights ->
  10x fewer conv FLOPs than materializing all expert convs)
  out[b] = conv2d(x[b], w_eff[b]) + b_eff[b]

Conv path runs in bf16 (x, w_eff) accumulating fp32 in PSUM. The conv is
9 shifted matmuls over the flat 58-wide grid; the PE array is quad-tiled
(2 samples on row halves x 2 chunk parities on column halves) so all four
64x64 quadrants stream concurrently.

w_eff is ALSO built on the PE: conv weights are host-packed as expert
pairs on partition halves, and per expert-pair a [128,64] stationary of
two stacked scaled identities contracts against them, accumulating the
10-expert weighted sum in PSUM. The mixture uses UNNORMALIZED softmax
numerators (cosine sims are in [-1,1] so exp() cannot overflow and the
max-subtraction is dropped); the 1/sum(exp) normalizer is folded into
the PSUM-evacuation scale. This keeps the serial routing tail to a
minimum of small cross-engine ops before the PE can start the conv.
"""
import sys

sys.path.insert(0, "/opt/trn_rl_repo")

import numpy as np
import ml_dtypes

import concourse.bass as bass
import concourse.mybir as mybir
from concourse.tile import TileContext

F32 = mybir.dt.float32
BF16 = mybir.dt.bfloat16
AF = mybir.ActivationFunctionType
ALU = mybir.AluOpType
AX = mybir.AxisListType

NCORES = 8
BLOC = 4           # samples per core
CIN = 64
COUT = 64
H = W = 58
HW = H * W         # 3364
OH = OW = 56
NB = 10            # experts
NG = 5             # expert pairs
EDIM = 64
RSIZE = 512
HID = 128
NTAP = 9
FD = NTAP * COUT   # 576
FDH = FD // 2      # 288 (PSUM bank-sized half)
GR = 7             # output rows per chunk
NGRP = 4           # 4 groups x (even chunk + odd chunk) x 7 rows = 56
NFREE = GR * W     # 406 <= 512 (one PSUM bank)
TAP_OFF = [dy * W + dx for dy in range(3) for dx in range(3)]
NWARM_MID = 5      # PE warmups between the tiny routing matmuls and conv

# blkb column layout (128 partitions, bf16): routing weights + identities
BB_RVT = 0                  # [128, 4c, 4b]
BB_W1 = BB_RVT + 16         # [128, 4c, 128m]
BB_W2 = BB_W1 + 512         # [128, 64]
BB_IP = BB_W2 + 64          # [128, 64] two stacked 64x64 identities
BB_D = BB_IP + 64           # 656 (bf16 cols; fp32 block follows as raw bytes)

# blk2 column layout (128 partitions, fp32)
B2_EMB = 0                  # [10, 64]
B2_CB = B2_EMB + 64         # [10, 64]
B2_ID = B2_CB + 64          # [16, 16]
B2_E4 = B2_ID + 16          # [4, 20]  E4[b',(b g)] = delta(b,b')
B2_ME = B2_E4 + 20          # [10, 20] mask_even[n,(b g)] = delta(n,2g)
B2_MO = B2_ME + 20          # [10, 20] mask_odd[n,(b g)]  = delta(n,2g+1)
B2_B1 = B2_MO + 20          # [128, 1]
B2_B2 = B2_B1 + 1           # [64, 1]
B2_D = B2_B2 + 1            # 206
BBT_D = BB_D + 2 * B2_D     # total bf16 cols of the merged block


def fix_sync_waits(nc, cap=2):
    """This walrus build allows at most `cap` sem waits per instruction.
    Splice same-engine NoOps carrying the excess waits right before any
    over-subscribed instruction (waits happen earlier => same semantics)."""
    uid = [0]
    for f in nc.m.functions:
        for blk in f.blocks:
            insts = blk.instructions  # live list
            i = 0
            while i < len(insts):
                inst = insts[i]
                si = inst.sync_info
                waits = list(si.on_wait) if si and si.on_wait else []
                icap = 1
                if len(waits) <= icap:
                    i += 1
                    continue
                keep, excess = waits[-icap:], waits[:-icap]
                for k in range(0, len(excess), icap):
                    nop = mybir.InstNoOp(
                        name=f"{inst.name}-wsplit{uid[0]}", ins=[], outs=[]
                    )
                    uid[0] += 1
                    nop.engine = inst.engine
                    nop.sync_info = mybir.SyncInfo(
                        on_wait=excess[k : k + icap], on_update=[]
                    )
                    nc.register_instruction(nop, overwrite=True)
                    insts.insert(i, nop)
                    i += 1
                inst.sync_info = mybir.SyncInfo(
                    on_wait=keep,
                    on_update=list(si.on_update) if si and si.on_update else [],
                )
                i += 1


def build():
    nc = bass.Bass(num_swdge_queues=4)
    blkb = nc.dram_tensor("blkb", [128, BBT_D], BF16, kind="ExternalInput")
    cwp = nc.dram_tensor("cwp", [128, NG, FD], BF16, kind="ExternalInput")
    x = nc.dram_tensor("x", [BLOC, CIN, HW], BF16, kind="ExternalInput")
    out = nc.dram_tensor("out", [BLOC, COUT, OH, OW], F32, kind="ExternalOutput")

    with TileContext(nc) as tc:
        with (
            tc.tile_pool(name="consts", bufs=1) as consts,
            tc.tile_pool(name="work", bufs=2) as work,
            tc.tile_pool(name="stage", bufs=3) as stage,
            tc.tile_pool(name="ps", bufs=2, space="PSUM") as pspool,
            tc.tile_pool(name="pswfsel", bufs=1, space="PSUM") as pswfsel,
            tc.tile_pool(name="psconv", bufs=2, space="PSUM") as psconv,
        ):
            # ---------- DMA in (all on sync hw queue, critical first) ----------
            bbt = consts.tile([128, BBT_D], BF16, tag="bbt")
            nc.sync.dma_start(out=bbt[:], in_=blkb[:])
            b2t = bbt[:, BB_D : BB_D + 2 * B2_D].bitcast(F32)
            cwt = consts.tile([128, NG, FD], BF16, tag="cwt")
            nc.sync.dma_start(out=cwt[:, 0:3], in_=cwp[:, 0:3])
            nc.sync.dma_start(out=cwt[:, 3:NG], in_=cwp[:, 3:NG])

            xt = []
            for j in range(2):
                t = consts.tile([128, HW + 4], BF16, tag=f"xt{j}")
                nc.vector.memset(t[:, HW : HW + 4], 0.0)
                xt.append(t)
            for b in range(BLOC):
                j, half = divmod(b, 2)
                nc.sync.dma_start(
                    out=xt[j][64 * half : 64 * half + 64, 0:HW], in_=x[b]
                )

            ones64 = consts.tile([EDIM, 1], F32, tag="ones64")
            nc.vector.memset(ones64[:], 1.0)
            # first table-func activation triggers the 1.3us ACT_TABLE_LOAD;
            # issue a dummy now so it overlaps the input DMA wait
            actwarm = consts.tile([1, 1], F32, tag="actwarm")
            nc.scalar.activation(out=actwarm[:], in_=ones64[0:1], func=AF.Exp)
            ones10_64 = consts.tile([NB, 64], F32, tag="ones10_64")
            nc.vector.memset(ones10_64[:], 1.0)
            ones4_128 = consts.tile([BLOC, 128], F32, tag="ones4_128")
            nc.vector.memset(ones4_128[:], 1.0)

            # views into the packed blocks
            rvT = bbt[:, BB_RVT : BB_RVT + 16].rearrange("p (c b) -> p c b", c=4)
            w1sb = bbt[:, BB_W1 : BB_W1 + 512].rearrange("p (c m) -> p c m", c=4)
            w2sb = bbt[:, BB_W2 : BB_W2 + 64]
            identPair = bbt[:, BB_IP : BB_IP + 64]
            embsb = b2t[0:NB, B2_EMB : B2_EMB + 64]
            cbsb = b2t[0:NB, B2_CB : B2_CB + 64]
            ident = b2t[0:16, B2_ID : B2_ID + 16]
            e4sb = b2t[0:4, B2_E4 : B2_E4 + 20]
            maskE = b2t[0:NB, B2_ME : B2_ME + 20]
            maskO = b2t[0:NB, B2_MO : B2_MO + 20]
            b1sb = b2t[:, B2_B1 : B2_B1 + 1]
            b2sb = b2t[0:EDIM, B2_B2 : B2_B2 + 1]

            # ---------- embedding norms (only needs blk2; runs first) ----------
            esq = work.tile([NB, EDIM], F32, tag="esq")
            nc.vector.tensor_mul(esq[:], embsb, embsb)
            ensq = work.tile([NB, 1], F32, tag="ensq")
            nc.vector.tensor_reduce(ensq[:], esq[:], axis=AX.X, op=ALU.add)
            eln = work.tile([NB, 1], F32, tag="eln")
            nc.scalar.activation(out=eln[:], in_=ensq[:], func=AF.Ln)
            einv = work.tile([NB, 1], F32, tag="einv")
            nc.scalar.activation(out=einv[:], in_=eln[:], func=AF.Exp, scale=-0.5)
            embn = work.tile([NB, EDIM], F32, tag="embn")
            nc.vector.tensor_scalar_mul(out=embn[:], in0=embsb, scalar1=einv[:])
            embnT_ps = pspool.tile([EDIM, NB], F32, tag="embp", bufs=1)
            nc.tensor.transpose(embnT_ps[:], embn[:], ident[0:NB, 0:NB])
            embnT = work.tile([EDIM, NB], F32, tag="embnT")
            nc.scalar.copy(out=embnT[:], in_=embnT_ps[:])

            # ---------- routing MLP (bf16 matmuls; rv pre-transposed) ----------
            h1 = pspool.tile([HID, BLOC], F32, tag="small")
            for c in range(4):
                nc.tensor.matmul(
                    h1[:], w1sb[:, c, :], rvT[:, c, :], start=(c == 0), stop=(c == 3)
                )
            h1r = work.tile([HID, BLOC], BF16, tag="h1r")
            nc.scalar.activation(
                out=h1r[:], in_=h1[:], func=AF.Relu, bias=b1sb, scale=1.0
            )
            rps = pspool.tile([EDIM, BLOC], F32, tag="small")
            nc.tensor.matmul(rps[:], w2sb, h1r[:], start=True, stop=True)
            rsb = work.tile([EDIM, BLOC], F32, tag="rsb")
            nc.scalar.activation(
                out=rsb[:], in_=rps[:], func=AF.Identity, bias=b2sb, scale=1.0
            )

            # ---------- r norms ----------
            rsq = work.tile([EDIM, BLOC], F32, tag="rsq")
            nc.scalar.activation(
                out=rsq[:], in_=rps[:], func=AF.Square, bias=b2sb, scale=1.0
            )
            nsq = pspool.tile([BLOC, 1], F32, tag="small")
            nc.tensor.matmul(nsq[:], rsq[:], ones64[:], start=True, stop=True)
            rln = work.tile([BLOC, 1], F32, tag="rln")
            nc.scalar.activation(out=rln[:], in_=nsq[:], func=AF.Ln)
            rinv = work.tile([BLOC, 1], F32, tag="rinv")
            nc.scalar.activation(out=rinv[:], in_=rln[:], func=AF.Exp, scale=-0.5)

            # ---------- similarity + unnormalized softmax numerators ----------
            # cosine sims are in [-1,1]: exp() cannot overflow, so skip the
            # max-subtraction and keep the numerators unnormalized; 1/sum is
            # applied at PSUM evacuation time.
            simps = pspool.tile([BLOC, NB], F32, tag="small")
            nc.tensor.matmul(simps[:], rsb[:], embnT[:], start=True, stop=True)
            ex = work.tile([BLOC, NB], F32, tag="ex")
            s = work.tile([BLOC, 1], F32, tag="s")
            nc.scalar.activation(
                out=ex[:], in_=simps[:], func=AF.Exp, scale=rinv[:],
                accum_out=s[:],
            )

            # ---------- per-(sample, expert-pair) scale table ----------
            # wfsel[p, (b g)] = ex[b, 2g] for p < 64, ex[b, 2g+1] for p >= 64
            wfx_ps = pspool.tile([NB, 20], F32, tag="small")
            nc.tensor.matmul(wfx_ps[:], ex[:], e4sb, start=True, stop=True)
            rhsE = work.tile([NB, 20], F32, tag="rhsE")
            nc.vector.tensor_mul(rhsE[:], wfx_ps[:], maskE)
            rhsO = work.tile([NB, 20], F32, tag="rhsO")
            nc.vector.tensor_mul(rhsO[:], wfx_ps[:], maskO)
            wfsel_ps = pswfsel.tile([128, 20], F32, tag="wfselp")
            nc.tensor.matmul(
                wfsel_ps[0:64], ones10_64[:], rhsE[:], start=True, stop=True,
                tile_position=(0, 0), skip_group_check=True,
            )
            nc.tensor.matmul(
                wfsel_ps[64:128], ones10_64[:], rhsO[:], start=True, stop=True,
                tile_position=(0, 64), skip_group_check=True,
            )
            wfsel = work.tile([128, 20], F32, tag="wfsel")
            nc.scalar.copy(out=wfsel[:], in_=wfsel_ps[:])

            # scaled-identity stationaries lhsw[b][g] = [ex[b,2g]*I; ex[b,2g+1]*I]
            # vector reads the PSUM table directly; scalar uses the SBUF copy.
            lhsw = [[None] * NG for _ in range(BLOC)]
            for b in range(BLOC):
                for g in range(NG):
                    lhsw[b][g] = work.tile(
                        [128, 64], BF16, tag=f"lh{b}_{g}", name=f"lh{b}_{g}"
                    )
            for b in (0, 2):
                for g in range(NG):
                    col = 5 * b + g
                    nc.vector.tensor_scalar_mul(
                        out=lhsw[b][g][:], in0=identPair,
                        scalar1=wfsel_ps[:, col : col + 1],
                    )
            for b in (1, 3):
                for g in range(NG):
                    col = 5 * b + g
                    nc.scalar.activation(
                        out=lhsw[b][g][:], in_=identPair, func=AF.Identity,
                        scale=wfsel[:, col : col + 1],
                    )

            # ---------- w_eff via PSUM-accumulated matmuls ----------
            # weff[0:64]  = weights for sample 2j   (PE row tile 0)
            # weff[64:128] = weights for sample 2j+1 (PE row tile 64)
            weffs = []
            for j in range(2):
                wpsA = psconv.tile([128, NFREE], F32, tag="psA")
                wpsB = psconv.tile([128, NFREE], F32, tag="psB")
                for g in range(NG):
                    st_, sp = (g == 0), (g == NG - 1)
                    for half, b in ((0, 2 * j), (1, 2 * j + 1)):
                        lo, hi = 64 * half, 64 * half + 64
                        nc.tensor.matmul(
                            wpsA[lo:hi, 0:FDH], lhsw[b][g], cwt[:, g, 0:FDH],
                            start=st_, stop=sp, tile_position=(0, 64 * half),
                            skip_group_check=True,
                        )
                        nc.tensor.matmul(
                            wpsB[lo:hi, 0:FDH], lhsw[b][g], cwt[:, g, FDH:FD],
                            start=st_, stop=sp, tile_position=(0, 64 * half),
                            skip_group_check=True,
                        )
                weff = work.tile(
                    [128, NTAP, COUT], BF16, tag=f"weff{j}", name=f"weff{j}"
                )
                wv = weff[:].rearrange("p t c -> p (t c)")
                nc.vector.tensor_copy(out=wv[:, 0:FDH], in_=wpsA[:, 0:FDH])
                nc.vector.tensor_copy(out=wv[:, FDH:FD], in_=wpsB[:, 0:FDH])
                weffs.append(weff)

            # ---------- off-critical: normalizers + effective bias ----------
            sinv = work.tile([BLOC, 1], F32, tag="sinv")
            nc.vector.reciprocal(sinv[:], s[:])
            wf = work.tile([BLOC, NB], F32, tag="wf")
            nc.vector.tensor_scalar_mul(out=wf[:], in0=ex[:], scalar1=sinv[:])
            diag4 = work.tile([BLOC, BLOC], F32, tag="diag4")
            nc.vector.tensor_scalar_mul(
                out=diag4[:], in0=ident[0:BLOC, 0:BLOC], scalar1=sinv[:]
            )
            wfT_ps = pspool.tile([NB, BLOC], F32, tag="embp", bufs=1)
            nc.tensor.transpose(wfT_ps[:], wf[:], ident[0:BLOC, 0:BLOC])
            wfT = work.tile([NB, BLOC], F32, tag="wfT")
            nc.scalar.copy(out=wfT[:], in_=wfT_ps[:])
            beff_ps = pspool.tile([128, BLOC], F32, tag="embp", bufs=1)
            nc.tensor.matmul(
                beff_ps[0:64], cbsb, wfT[:], start=True, stop=True,
                tile_position=(0, 0),
            )
            nc.tensor.matmul(
                beff_ps[64:128], cbsb, wfT[:], start=True, stop=True,
                tile_position=(0, 64), skip_group_check=True,
            )
            beff2 = work.tile([128, BLOC], F32, tag="beff2")
            nc.scalar.copy(out=beff2[:], in_=beff_ps[:])
            sinvb_ps = pspool.tile([128, BLOC], F32, tag="embp", bufs=1)
            nc.tensor.matmul(
                sinvb_ps[:], ones4_128[:], diag4[:], start=True, stop=True
            )
            sinvb = work.tile([128, BLOC], F32, tag="sinvb")
            nc.scalar.copy(out=sinvb[:], in_=sinvb_ps[:])

            # ---------- conv: quad-tiled 9-tap shifted matmuls ----------
            for j in range(2):
                weff = weffs[j]
                for g in range(NGRP):
                    h_e = 2 * g * GR       # even chunk start row
                    h_o = h_e + GR         # odd chunk start row
                    psA = psconv.tile([128, NFREE], F32, tag="psA")
                    psB = psconv.tile([128, NFREE], F32, tag="psB")
                    for t in range(NTAP):
                        oe = h_e * W + TAP_OFF[t]
                        oo = h_o * W + TAP_OFF[t]
                        st_, sp = (t == 0), (t == NTAP - 1)
                        nc.tensor.matmul(
                            psA[0:64], weff[0:64, t, :], xt[j][0:64, oe : oe + NFREE],
                            start=st_, stop=sp, tile_position=(0, 0),
                            skip_group_check=True,
                        )
                        nc.tensor.matmul(
                            psA[64:128], weff[0:64, t, :], xt[j][0:64, oo : oo + NFREE],
                            start=st_, stop=sp, tile_position=(0, 64),
                            skip_group_check=True,
                        )
                        nc.tensor.matmul(
                            psB[0:64], weff[64:128, t, :], xt[j][64:128, oe : oe + NFREE],
                            start=st_, stop=sp, tile_position=(64, 0),
                            skip_group_check=True,
                        )
                        nc.tensor.matmul(
                            psB[64:128], weff[64:128, t, :], xt[j][64:128, oo : oo + NFREE],
                            start=st_, stop=sp, tile_position=(64, 64),
                            skip_group_check=True,
                        )
                    # evacuate both parities in one op per sample (vector),
                    # applying the softmax normalizer and bias; trim 58->56;
                    # store each row-parity immediately (sync + scalar queues)
                    for half, ps in ((0, psA), (1, psB)):
                        b = 2 * j + half
                        st = stage.tile(
                            [128, GR, OW], F32,
                            tag=f"stg{half}", name=f"stg{half}",
                        )
                        pv = ps[:].rearrange("p (r w) -> p r w", w=W)[:, :, 0:OW]
                        nc.vector.tensor_scalar(
                            out=st[:], in0=pv,
                            scalar1=sinvb[:, b : b + 1],
                            scalar2=beff2[:, b : b + 1],
                            op0=ALU.mult, op1=ALU.add,
                        )
                        dv = out[b].rearrange(
                            "c (G g2 r) w -> g2 c G r w", G=NGRP, g2=2
                        )
                        nc.sync.dma_start(out=dv[0, :, g], in_=st[0:64])
                        nc.scalar.dma_start(out=dv[1, :, g], in_=st[64:128])

    fix_sync_waits(nc)
    return nc


_NC = None


def _get_nc():
    global _NC
    if _NC is None:
        _NC = build()
    return _NC


def make_in_maps(inputs):
    bf16 = ml_dtypes.bfloat16
    x = np.asarray(inputs["x"], dtype=np.float32).reshape(32, CIN, HW)
    rvec = np.asarray(inputs["routing_vector"], dtype=np.float32)
    W1 = np.asarray(inputs["W1"], dtype=np.float32)
    b1 = np.asarray(inputs["b1"], dtype=np.float32)
    W2 = np.asarray(inputs["W2"], dtype=np.float32)
    b2 = np.asarray(inputs["b2"], dtype=np.float32)
    emb = np.asarray(inputs["emb"], dtype=np.float32)
    conv_w = np.asarray(inputs["conv_w"], dtype=np.float32)
    conv_b = np.asarray(inputs["conv_b"], dtype=np.float32)

    # conv_w[n, co, ci, ky, kx] -> cwp[(n%2)*64+ci, n//2, (ky kx)*co], bf16
    cwpt = conv_w.transpose(2, 0, 3, 4, 1).reshape(CIN, NB, FD)   # [ci, n, tc]
    cwpa = np.zeros((128, NG, FD), np.float32)
    for g in range(NG):
        cwpa[0:64, g] = cwpt[:, 2 * g]
        cwpa[64:128, g] = cwpt[:, 2 * g + 1]
    cwpa = np.ascontiguousarray(cwpa).astype(bf16)

    # blkb (bf16): per-core rvT + routing weights + stacked identity pair
    w1blk = W1.reshape(4, 128, HID).transpose(1, 0, 2).reshape(128, 512)
    blkb_shared = np.zeros((128, BB_D), np.float32)
    blkb_shared[:, BB_W1 : BB_W1 + 512] = w1blk
    blkb_shared[:, BB_W2 : BB_W2 + 64] = W2
    eye64 = np.eye(64, dtype=np.float32)
    blkb_shared[0:64, BB_IP : BB_IP + 64] = eye64
    blkb_shared[64:128, BB_IP : BB_IP + 64] = eye64

    # blk2 (fp32): emb, conv bias, identity, wfsel helper constants, biases
    blk2a = np.zeros((128, B2_D), np.float32)
    blk2a[0:NB, B2_EMB : B2_EMB + 64] = emb
    blk2a[0:NB, B2_CB : B2_CB + 64] = conv_b
    blk2a[0:16, B2_ID : B2_ID + 16] = np.eye(16, dtype=np.float32)
    for b in range(BLOC):
        for g in range(NG):
            blk2a[b, B2_E4 + 5 * b + g] = 1.0
            blk2a[2 * g, B2_ME + 5 * b + g] = 1.0
            blk2a[2 * g + 1, B2_MO + 5 * b + g] = 1.0
    blk2a[:, B2_B1] = b1
    blk2a[0:EDIM, B2_B2] = b2

    xb = x.astype(bf16)
    in_maps = []
    for c in range(NCORES):
        blkba = blkb_shared.copy()
        rvc = rvec[BLOC * c : BLOC * (c + 1)]          # [4, 512]
        # rvT[p, c, b] = rv[b, 128c + p]
        rvt = rvc.T.reshape(4, 128, BLOC).transpose(1, 0, 2).reshape(128, 16)
        blkba[:, BB_RVT : BB_RVT + 16] = rvt
        merged = np.zeros((128, 2 * BBT_D), np.uint8)
        merged[:, : 2 * BB_D] = (
            blkba.astype(bf16).view(np.uint8).reshape(128, 2 * BB_D)
        )
        merged[:, 2 * BB_D :] = blk2a.view(np.uint8).reshape(128, 4 * B2_D)
        in_maps.append(
            {
                "blkb": np.ascontiguousarray(merged).view(bf16),
                "cwp": cwpa,
                "x": np.ascontiguousarray(xb[BLOC * c : BLOC * (c + 1)]),
            }
        )
    return in_maps


def kernel(**inputs):
    from concourse.bass_utils import run_bass_kernel_spmd

    nc = _get_nc()
    in_maps = make_in_maps(inputs)
    res = run_bass_kernel_spmd(nc, in_maps, core_ids=list(range(NCORES)))
    return np.concatenate([r["out"] for r in res.results], axis=0)


# revision 18
# speedup vs baseline: 1.0015x; 1.0015x over previous
"""MoE routing layer on 8 Trainium2 NeuronCores (data-parallel over batch).

Per core (4 samples):
  routing MLP -> cosine sim vs embeddings -> softmax weights wf[4,10]
  w_eff[b] = sum_n wf[b,n] * conv_w[n]  (conv is linear in weights ->
  10x fewer conv FLOPs than materializing all expert convs)
  out[b] = conv2d(x[b], w_eff[b]) + b_eff[b]

Conv path runs in bf16 (x, w_eff) accumulating fp32 in PSUM. The conv is
9 shifted matmuls over the flat 58-wide grid; the PE array is quad-tiled
(2 samples on row halves x 2 chunk parities on column halves) so all four
64x64 quadrants stream concurrently.

w_eff is ALSO built on the PE: conv weights are host-packed as expert
pairs on partition halves, and per expert-pair a [128,64] stationary of
two stacked scaled identities contracts against them, accumulating the
10-expert weighted sum in PSUM. The mixture uses UNNORMALIZED softmax
numerators (cosine sims are in [-1,1] so exp() cannot overflow and the
max-subtraction is dropped); the 1/sum(exp) normalizer is folded into
the PSUM-evacuation scale. This keeps the serial routing tail to a
minimum of small cross-engine ops before the PE can start the conv.
"""
import sys

sys.path.insert(0, "/opt/trn_rl_repo")

import numpy as np
import ml_dtypes

import concourse.bass as bass
import concourse.mybir as mybir
from concourse.tile import TileContext

F32 = mybir.dt.float32
BF16 = mybir.dt.bfloat16
AF = mybir.ActivationFunctionType
ALU = mybir.AluOpType
AX = mybir.AxisListType

NCORES = 8
BLOC = 4           # samples per core
CIN = 64
COUT = 64
H = W = 58
HW = H * W         # 3364
OH = OW = 56
NB = 10            # experts
NG = 5             # expert pairs
EDIM = 64
RSIZE = 512
HID = 128
NTAP = 9
FD = NTAP * COUT   # 576
FDH = FD // 2      # 288 (PSUM bank-sized half)
GR = 7             # output rows per chunk
NGRP = 4           # 4 groups x (even chunk + odd chunk) x 7 rows = 56
NFREE = GR * W     # 406 <= 512 (one PSUM bank)
TAP_OFF = [dy * W + dx for dy in range(3) for dx in range(3)]
NWARM_MID = 5      # PE warmups between the tiny routing matmuls and conv

# blkb column layout (128 partitions, bf16): routing weights + identities
BB_RVT = 0                  # [128, 4c, 4b]
BB_W1 = BB_RVT + 16         # [128, 4c, 128m]
BB_W2 = BB_W1 + 512         # [128, 64]
BB_IP = BB_W2 + 64          # [128, 64] two stacked 64x64 identities
BB_D = BB_IP + 64           # 656 (bf16 cols; fp32 block follows as raw bytes)

# blk2 column layout (128 partitions, fp32)
B2_EMB = 0                  # [10, 64]
B2_CB = B2_EMB + 64         # [10, 64]
B2_ID = B2_CB + 64          # [16, 16]
B2_E4 = B2_ID + 16          # [4, 20]  E4[b',(b g)] = delta(b,b')
B2_ME = B2_E4 + 20          # [10, 20] mask_even[n,(b g)] = delta(n,2g)
B2_MO = B2_ME + 20          # [10, 20] mask_odd[n,(b g)]  = delta(n,2g+1)
B2_B1 = B2_MO + 20          # [128, 1]
B2_B2 = B2_B1 + 1           # [64, 1]
B2_D = B2_B2 + 1            # 206
BBT_D = BB_D + 2 * B2_D     # total bf16 cols of the merged block


def fix_sync_waits(nc, cap=2):
    """This walrus build allows at most `cap` sem waits per instruction.
    Splice same-engine NoOps carrying the excess waits right before any
    over-subscribed instruction (waits happen earlier => same semantics)."""
    uid = [0]
    for f in nc.m.functions:
        for blk in f.blocks:
            insts = blk.instructions  # live list
            i = 0
            while i < len(insts):
                inst = insts[i]
                si = inst.sync_info
                waits = list(si.on_wait) if si and si.on_wait else []
                icap = 1
                if len(waits) <= icap:
                    i += 1
                    continue
                keep, excess = waits[-icap:], waits[:-icap]
                for k in range(0, len(excess), icap):
                    nop = mybir.InstNoOp(
                        name=f"{inst.name}-wsplit{uid[0]}", ins=[], outs=[]
                    )
                    uid[0] += 1
                    nop.engine = inst.engine
                    nop.sync_info = mybir.SyncInfo(
                        on_wait=excess[k : k + icap], on_update=[]
                    )
                    nc.register_instruction(nop, overwrite=True)
                    insts.insert(i, nop)
                    i += 1
                inst.sync_info = mybir.SyncInfo(
                    on_wait=keep,
                    on_update=list(si.on_update) if si and si.on_update else [],
                )
                i += 1


def build():
    nc = bass.Bass(num_swdge_queues=4)
    blkb = nc.dram_tensor("blkb", [128, BBT_D], BF16, kind="ExternalInput")
    cwp = nc.dram_tensor("cwp", [128, NG, FD], BF16, kind="ExternalInput")
    x = nc.dram_tensor("x", [BLOC, CIN, HW], BF16, kind="ExternalInput")
    out = nc.dram_tensor("out", [BLOC, COUT, OH, OW], F32, kind="ExternalOutput")

    with TileContext(nc) as tc:
        with (
            tc.tile_pool(name="consts", bufs=1) as consts,
            tc.tile_pool(name="work", bufs=2) as work,
            tc.tile_pool(name="stage", bufs=3) as stage,
            tc.tile_pool(name="ps", bufs=2, space="PSUM") as pspool,
            tc.tile_pool(name="pswfsel", bufs=1, space="PSUM") as pswfsel,
            tc.tile_pool(name="psconv", bufs=2, space="PSUM") as psconv,
        ):
            # ---------- DMA in (all on sync hw queue, critical first) ----------
            bbt = consts.tile([128, BBT_D], BF16, tag="bbt")
            nc.sync.dma_start(out=bbt[:], in_=blkb[:])
            b2t = bbt[:, BB_D : BB_D + 2 * B2_D].bitcast(F32)
            cwt = consts.tile([128, NG, FD], BF16, tag="cwt")
            nc.sync.dma_start(out=cwt[:, 0:3], in_=cwp[:, 0:3])
            nc.sync.dma_start(out=cwt[:, 3:NG], in_=cwp[:, 3:NG])

            xt = []
            for j in range(2):
                t = consts.tile([128, HW + 4], BF16, tag=f"xt{j}")
                nc.vector.memset(t[:, HW : HW + 4], 0.0)
                xt.append(t)
            for b in range(BLOC):
                j, half = divmod(b, 2)
                nc.sync.dma_start(
                    out=xt[j][64 * half : 64 * half + 64, 0:HW], in_=x[b]
                )

            ones64 = consts.tile([EDIM, 1], F32, tag="ones64")
            nc.vector.memset(ones64[:], 1.0)
            ones10_64 = consts.tile([NB, 64], F32, tag="ones10_64")
            nc.vector.memset(ones10_64[:], 1.0)
            ones4_128 = consts.tile([BLOC, 128], F32, tag="ones4_128")
            nc.vector.memset(ones4_128[:], 1.0)
            # first table-func activation triggers the 1.3us ACT_TABLE_LOAD;
            # issue a dummy now so it overlaps the input DMA wait
            actwarm = consts.tile([1, 1], F32, tag="actwarm")
            nc.scalar.activation(out=actwarm[:], in_=ones64[0:1], func=AF.Exp)

            # views into the packed blocks
            rvT = bbt[:, BB_RVT : BB_RVT + 16].rearrange("p (c b) -> p c b", c=4)
            w1sb = bbt[:, BB_W1 : BB_W1 + 512].rearrange("p (c m) -> p c m", c=4)
            w2sb = bbt[:, BB_W2 : BB_W2 + 64]
            identPair = bbt[:, BB_IP : BB_IP + 64]
            embsb = b2t[0:NB, B2_EMB : B2_EMB + 64]
            cbsb = b2t[0:NB, B2_CB : B2_CB + 64]
            ident = b2t[0:16, B2_ID : B2_ID + 16]
            e4sb = b2t[0:4, B2_E4 : B2_E4 + 20]
            maskE = b2t[0:NB, B2_ME : B2_ME + 20]
            maskO = b2t[0:NB, B2_MO : B2_MO + 20]
            b1sb = b2t[:, B2_B1 : B2_B1 + 1]
            b2sb = b2t[0:EDIM, B2_B2 : B2_B2 + 1]

            # ---------- embedding norms (independent of rv; runs early) -------
            esq = work.tile([NB, EDIM], F32, tag="esq")
            nc.vector.tensor_mul(esq[:], embsb, embsb)
            ensq = work.tile([NB, 1], F32, tag="ensq")
            nc.vector.tensor_reduce(ensq[:], esq[:], axis=AX.X, op=ALU.add)
            eln = work.tile([NB, 1], F32, tag="eln")
            nc.scalar.activation(out=eln[:], in_=ensq[:], func=AF.Ln)
            einv = work.tile([NB, 1], F32, tag="einv")
            nc.scalar.activation(out=einv[:], in_=eln[:], func=AF.Exp, scale=-0.5)
            embn = work.tile([NB, EDIM], F32, tag="embn")
            nc.vector.tensor_scalar_mul(out=embn[:], in0=embsb, scalar1=einv[:])
            embnT_ps = pspool.tile([EDIM, NB], F32, tag="embp", bufs=1)
            nc.tensor.transpose(embnT_ps[:], embn[:], ident[0:NB, 0:NB])
            embnT = work.tile([EDIM, NB], F32, tag="embnT")
            nc.scalar.copy(out=embnT[:], in_=embnT_ps[:])

            # ---------- routing MLP (bf16 matmuls; rv pre-transposed) ----------
            h1 = pspool.tile([HID, BLOC], F32, tag="small")
            for c in range(4):
                nc.tensor.matmul(
                    h1[:], w1sb[:, c, :], rvT[:, c, :], start=(c == 0), stop=(c == 3)
                )
            h1r = work.tile([HID, BLOC], BF16, tag="h1r")
            nc.scalar.activation(
                out=h1r[:], in_=h1[:], func=AF.Relu, bias=b1sb, scale=1.0
            )
            rps = pspool.tile([EDIM, BLOC], F32, tag="small")
            nc.tensor.matmul(rps[:], w2sb, h1r[:], start=True, stop=True)
            rsb = work.tile([EDIM, BLOC], F32, tag="rsb")
            nc.scalar.activation(
                out=rsb[:], in_=rps[:], func=AF.Identity, bias=b2sb, scale=1.0
            )

            # ---------- r norms (rsq straight off the PSUM result) ----------
            rsq = work.tile([EDIM, BLOC], F32, tag="rsq")
            nc.scalar.activation(
                out=rsq[:], in_=rps[:], func=AF.Square, bias=b2sb, scale=1.0
            )
            nsq = pspool.tile([BLOC, 1], F32, tag="small")
            nc.tensor.matmul(nsq[:], rsq[:], ones64[:], start=True, stop=True)
            rln = work.tile([BLOC, 1], F32, tag="rln")
            nc.scalar.activation(out=rln[:], in_=nsq[:], func=AF.Ln)
            rinv = work.tile([BLOC, 1], F32, tag="rinv")
            nc.scalar.activation(out=rinv[:], in_=rln[:], func=AF.Exp, scale=-0.5)

            # ---------- similarity + unnormalized softmax numerators ----------
            # cosine sims are in [-1,1]: exp() cannot overflow, so skip the
            # max-subtraction; 1/sum is applied at PSUM evacuation time.
            simps = pspool.tile([BLOC, NB], F32, tag="small")
            nc.tensor.matmul(simps[:], rsb[:], embnT[:], start=True, stop=True)
            ex = work.tile([BLOC, NB], F32, tag="ex")
            s = work.tile([BLOC, 1], F32, tag="s")
            nc.scalar.activation(
                out=ex[:], in_=simps[:], func=AF.Exp, scale=rinv[:],
                accum_out=s[:],
            )

            # ---------- per-(sample, expert-pair) scale table ----------
            # wfsel[p, (b g)] = ex[b, 2g] for p < 64, ex[b, 2g+1] for p >= 64
            wfx_ps = pspool.tile([NB, 20], F32, tag="small")
            nc.tensor.matmul(wfx_ps[:], ex[:], e4sb, start=True, stop=True)
            exT_ps = pspool.tile([NB, BLOC], F32, tag="embp", bufs=1)
            nc.tensor.transpose(exT_ps[:], ex[:], ident[0:BLOC, 0:BLOC])
            exT = work.tile([NB, BLOC], F32, tag="exT")
            nc.scalar.copy(out=exT[:], in_=exT_ps[:])
            rhsE = work.tile([NB, 20], F32, tag="rhsE")
            nc.vector.tensor_mul(rhsE[:], wfx_ps[:], maskE)
            rhsO = work.tile([NB, 20], F32, tag="rhsO")
            nc.vector.tensor_mul(rhsO[:], wfx_ps[:], maskO)
            wfsel_ps = pswfsel.tile([128, 20], F32, tag="wfselp")
            nc.tensor.matmul(
                wfsel_ps[0:64], ones10_64[:], rhsE[:], start=True, stop=True,
                tile_position=(0, 0), skip_group_check=True,
            )
            nc.tensor.matmul(
                wfsel_ps[64:128], ones10_64[:], rhsO[:], start=True, stop=True,
                tile_position=(0, 64), skip_group_check=True,
            )
            wfsel = work.tile([128, 20], F32, tag="wfsel")
            nc.scalar.copy(out=wfsel[:], in_=wfsel_ps[:])

            # ---------- normalizer broadcast + unnormalized bias ----------
            sinv = work.tile([BLOC, 1], F32, tag="sinv")
            nc.vector.reciprocal(sinv[:], s[:])
            diag4 = work.tile([BLOC, BLOC], F32, tag="diag4")
            nc.vector.tensor_scalar_mul(
                out=diag4[:], in0=ident[0:BLOC, 0:BLOC], scalar1=sinv[:]
            )
            sinvb_ps = pspool.tile([128, BLOC], F32, tag="embp", bufs=1)
            nc.tensor.matmul(
                sinvb_ps[:], ones4_128[:], diag4[:], start=True, stop=True
            )
            sinvb = work.tile([128, BLOC], F32, tag="sinvb")
            nc.scalar.copy(out=sinvb[:], in_=sinvb_ps[:])
            # beffU = cb^T ex (unnormalized); evacuation computes (pv+bU)/s
            beff_ps = pspool.tile([128, BLOC], F32, tag="embp", bufs=1)
            nc.tensor.matmul(
                beff_ps[0:64], cbsb, exT[:], start=True, stop=True,
                tile_position=(0, 0),
            )
            nc.tensor.matmul(
                beff_ps[64:128], cbsb, exT[:], start=True, stop=True,
                tile_position=(0, 64), skip_group_check=True,
            )
            beffU = work.tile([128, BLOC], F32, tag="beffU")
            nc.scalar.copy(out=beffU[:], in_=beff_ps[:])

            # scaled-identity stationaries lhsw[b][g]=[ex[b,2g]*I; ex[b,2g+1]*I]
            # vector builds even samples off the PSUM table, scalar the odd
            # samples off the SBUF copy
            lhsw = [[None] * NG for _ in range(BLOC)]
            for b in range(BLOC):
                for g in range(NG):
                    lhsw[b][g] = work.tile(
                        [128, 64], BF16, tag=f"lh{b}_{g}", name=f"lh{b}_{g}"
                    )
            for b in (0, 2):
                for g in range(NG):
                    col = 5 * b + g
                    nc.vector.tensor_scalar_mul(
                        out=lhsw[b][g][:], in0=identPair,
                        scalar1=wfsel_ps[:, col : col + 1],
                    )
            for b in (1, 3):
                for g in range(NG):
                    col = 5 * b + g
                    nc.scalar.activation(
                        out=lhsw[b][g][:], in_=identPair, func=AF.Identity,
                        scale=wfsel[:, col : col + 1],
                    )

            # beffN = beffU/s for the scalar-side evacuation bias
            beffN = work.tile([128, BLOC], F32, tag="beffN")
            nc.vector.tensor_mul(beffN[:], beffU[:], sinvb[:])

            # ---------- w_eff via PSUM-accumulated matmuls ----------
            # weff[0:64]  = weights for sample 2j   (PE row tile 0)
            # weff[64:128] = weights for sample 2j+1 (PE row tile 64)
            weffs = []
            for j in range(2):
                wpsA = psconv.tile([128, NFREE], F32, tag="psA")
                wpsB = psconv.tile([128, NFREE], F32, tag="psB")
                for g in range(NG):
                    st_, sp = (g == 0), (g == NG - 1)
                    for half, b in ((0, 2 * j), (1, 2 * j + 1)):
                        lo, hi = 64 * half, 64 * half + 64
                        nc.tensor.matmul(
                            wpsA[lo:hi, 0:FDH], lhsw[b][g], cwt[:, g, 0:FDH],
                            start=st_, stop=sp, tile_position=(0, 64 * half),
                            skip_group_check=True,
                        )
                        nc.tensor.matmul(
                            wpsB[lo:hi, 0:FDH], lhsw[b][g], cwt[:, g, FDH:FD],
                            start=st_, stop=sp, tile_position=(0, 64 * half),
                            skip_group_check=True,
                        )
                weff = work.tile(
                    [128, NTAP, COUT], BF16, tag=f"weff{j}", name=f"weff{j}"
                )
                wv = weff[:].rearrange("p t c -> p (t c)")
                nc.vector.tensor_copy(out=wv[:, 0:FDH], in_=wpsA[:, 0:FDH])
                nc.vector.tensor_copy(out=wv[:, FDH:FD], in_=wpsB[:, 0:FDH])
                weffs.append(weff)

            # ---------- conv: quad-tiled 9-tap shifted matmuls ----------
            for j in range(2):
                weff = weffs[j]
                for g in range(NGRP):
                    h_e = 2 * g * GR       # even chunk start row
                    h_o = h_e + GR         # odd chunk start row
                    psA = psconv.tile([128, NFREE], F32, tag="psA")
                    psB = psconv.tile([128, NFREE], F32, tag="psB")
                    for t in range(NTAP):
                        oe = h_e * W + TAP_OFF[t]
                        oo = h_o * W + TAP_OFF[t]
                        st_, sp = (t == 0), (t == NTAP - 1)
                        nc.tensor.matmul(
                            psA[0:64], weff[0:64, t, :], xt[j][0:64, oe : oe + NFREE],
                            start=st_, stop=sp, tile_position=(0, 0),
                            skip_group_check=True,
                        )
                        nc.tensor.matmul(
                            psA[64:128], weff[0:64, t, :], xt[j][0:64, oo : oo + NFREE],
                            start=st_, stop=sp, tile_position=(0, 64),
                            skip_group_check=True,
                        )
                        nc.tensor.matmul(
                            psB[0:64], weff[64:128, t, :], xt[j][64:128, oe : oe + NFREE],
                            start=st_, stop=sp, tile_position=(64, 0),
                            skip_group_check=True,
                        )
                        nc.tensor.matmul(
                            psB[64:128], weff[64:128, t, :], xt[j][64:128, oo : oo + NFREE],
                            start=st_, stop=sp, tile_position=(64, 64),
                            skip_group_check=True,
                        )
                    # evacuate: sample-even PSUM on scalar (ACT scale+bias),
                    # sample-odd on vector ((pv + beffU) * 1/s); then store
                    # each row-parity (sync gets 24 dispatches, scalar 8)
                    for half, ps in ((0, psA), (1, psB)):
                        b = 2 * j + half
                        st = stage.tile(
                            [128, GR, OW], F32,
                            tag=f"stg{half}", name=f"stg{half}",
                        )
                        pv = ps[:].rearrange("p (r w) -> p r w", w=W)[:, :, 0:OW]
                        if half == 0:
                            nc.scalar.activation(
                                out=st[:], in_=pv, func=AF.Identity,
                                bias=beffN[:, b : b + 1],
                                scale=sinvb[:, b : b + 1],
                            )
                        else:
                            nc.vector.tensor_scalar(
                                out=st[:], in0=pv,
                                scalar1=beffU[:, b : b + 1],
                                scalar2=sinvb[:, b : b + 1],
                                op0=ALU.add, op1=ALU.mult,
                            )
                        dv = out[b].rearrange(
                            "c (G g2 r) w -> g2 c G r w", G=NGRP, g2=2
                        )
                        nc.sync.dma_start(out=dv[0, :, g], in_=st[0:64])
                        if j == 0:
                            nc.scalar.dma_start(out=dv[1, :, g], in_=st[64:128])
                        else:
                            nc.sync.dma_start(out=dv[1, :, g], in_=st[64:128])

    fix_sync_waits(nc)
    return nc


_NC = None


def _get_nc():
    global _NC
    if _NC is None:
        _NC = build()
    return _NC


def make_in_maps(inputs):
    bf16 = ml_dtypes.bfloat16
    x = np.asarray(inputs["x"], dtype=np.float32).reshape(32, CIN, HW)
    rvec = np.asarray(inputs["routing_vector"], dtype=np.float32)
    W1 = np.asarray(inputs["W1"], dtype=np.float32)
    b1 = np.asarray(inputs["b1"], dtype=np.float32)
    W2 = np.asarray(inputs["W2"], dtype=np.float32)
    b2 = np.asarray(inputs["b2"], dtype=np.float32)
    emb = np.asarray(inputs["emb"], dtype=np.float32)
    conv_w = np.asarray(inputs["conv_w"], dtype=np.float32)
    conv_b = np.asarray(inputs["conv_b"], dtype=np.float32)

    # conv_w[n, co, ci, ky, kx] -> cwp[(n%2)*64+ci, n//2, (ky kx)*co], bf16
    cwpt = conv_w.transpose(2, 0, 3, 4, 1).reshape(CIN, NB, FD)   # [ci, n, tc]
    cwpa = np.zeros((128, NG, FD), np.float32)
    for g in range(NG):
        cwpa[0:64, g] = cwpt[:, 2 * g]
        cwpa[64:128, g] = cwpt[:, 2 * g + 1]
    cwpa = np.ascontiguousarray(cwpa).astype(bf16)

    # blkb (bf16): per-core rvT + routing weights + stacked identity pair
    w1blk = W1.reshape(4, 128, HID).transpose(1, 0, 2).reshape(128, 512)
    blkb_shared = np.zeros((128, BB_D), np.float32)
    blkb_shared[:, BB_W1 : BB_W1 + 512] = w1blk
    blkb_shared[:, BB_W2 : BB_W2 + 64] = W2
    eye64 = np.eye(64, dtype=np.float32)
    blkb_shared[0:64, BB_IP : BB_IP + 64] = eye64
    blkb_shared[64:128, BB_IP : BB_IP + 64] = eye64

    # blk2 (fp32): emb, conv bias, identity, wfsel helper constants, biases
    blk2a = np.zeros((128, B2_D), np.float32)
    blk2a[0:NB, B2_EMB : B2_EMB + 64] = emb
    blk2a[0:NB, B2_CB : B2_CB + 64] = conv_b
    blk2a[0:16, B2_ID : B2_ID + 16] = np.eye(16, dtype=np.float32)
    for b in range(BLOC):
        for g in range(NG):
            blk2a[b, B2_E4 + 5 * b + g] = 1.0
            blk2a[2 * g, B2_ME + 5 * b + g] = 1.0
            blk2a[2 * g + 1, B2_MO + 5 * b + g] = 1.0
    blk2a[:, B2_B1] = b1
    blk2a[0:EDIM, B2_B2] = b2

    xb = x.astype(bf16)
    in_maps = []
    for c in range(NCORES):
        blkba = blkb_shared.copy()
        rvc = rvec[BLOC * c : BLOC * (c + 1)]          # [4, 512]
        # rvT[p, c, b] = rv[b, 128c + p]
        rvt = rvc.T.reshape(4, 128, BLOC).transpose(1, 0, 2).reshape(128, 16)
        blkba[:, BB_RVT : BB_RVT + 16] = rvt
        merged = np.zeros((128, 2 * BBT_D), np.uint8)
        merged[:, : 2 * BB_D] = (
            blkba.astype(bf16).view(np.uint8).reshape(128, 2 * BB_D)
        )
        merged[:, 2 * BB_D :] = blk2a.view(np.uint8).reshape(128, 4 * B2_D)
        in_maps.append(
            {
                "blkb": np.ascontiguousarray(merged).view(bf16),
                "cwp": cwpa,
                "x": np.ascontiguousarray(xb[BLOC * c : BLOC * (c + 1)]),
            }
        )
    return in_maps


def kernel(**inputs):
    from concourse.bass_utils import run_bass_kernel_spmd

    nc = _get_nc()
    in_maps = make_in_maps(inputs)
    res = run_bass_kernel_spmd(nc, in_maps, core_ids=list(range(NCORES)))
    return np.concatenate([r["out"] for r in res.results], axis=0)


# revision 19
# speedup vs baseline: 1.1927x; 1.1909x over previous
"""MoE routing layer on 8 Trainium2 NeuronCores (data-parallel over batch).

Per core (4 samples):
  routing MLP -> cosine sim vs embeddings -> softmax weights wf[4,10]
  w_eff[b] = sum_n wf[b,n] * conv_w[n]  (conv is linear in weights ->
  10x fewer conv FLOPs than materializing all expert convs)
  out[b] = conv2d(x[b], w_eff[b]) + b_eff[b]

Conv path runs in bf16 (x, w_eff) accumulating fp32 in PSUM. The conv is
9 shifted matmuls over the flat 58-wide grid; the PE array is quad-tiled
(2 samples on row halves x 2 chunk parities on column halves) so all four
64x64 quadrants stream concurrently.

w_eff is ALSO built on the PE: conv weights are host-packed as expert
pairs on partition halves, and per expert-pair a [128,64] stationary of
two stacked scaled identities contracts against them, accumulating the
10-expert weighted sum in PSUM. The mixture uses UNNORMALIZED softmax
numerators (cosine sims are in [-1,1] so exp() cannot overflow and the
max-subtraction is dropped); the 1/sum(exp) normalizer is folded into
the PSUM-evacuation scale. This keeps the serial routing tail to a
minimum of small cross-engine ops before the PE can start the conv.
"""
import sys

sys.path.insert(0, "/opt/trn_rl_repo")

import numpy as np
import ml_dtypes

import concourse.bass as bass
import concourse.mybir as mybir
from concourse.tile import TileContext

F32 = mybir.dt.float32
BF16 = mybir.dt.bfloat16
AF = mybir.ActivationFunctionType
ALU = mybir.AluOpType
AX = mybir.AxisListType

NCORES = 8
BLOC = 4           # samples per core
CIN = 64
COUT = 64
H = W = 58
HW = H * W         # 3364
OH = OW = 56
NB = 10            # experts
NG = 5             # expert pairs
EDIM = 64
RSIZE = 512
HID = 128
NTAP = 9
FD = NTAP * COUT   # 576
FDH = FD // 2      # 288 (PSUM bank-sized half)
GR = 7             # output rows per chunk
NGRP = 4           # 4 groups x (even chunk + odd chunk) x 7 rows = 56
NFREE = GR * W     # 406 <= 512 (one PSUM bank)
TAP_OFF = [dy * W + dx for dy in range(3) for dx in range(3)]
NWARM_MID = 5      # PE warmups between the tiny routing matmuls and conv

# blkb column layout (128 partitions, bf16): routing weights + identities
BB_RVT = 0                  # [128, 4c, 4b]
BB_W1 = BB_RVT + 16         # [128, 4c, 128m]
BB_W2 = BB_W1 + 512         # [128, 64]
BB_IP = BB_W2 + 64          # [128, 64] two stacked 64x64 identities
BB_D = BB_IP + 64           # 656 (bf16 cols; fp32 block follows as raw bytes)

# blk2 column layout (128 partitions, fp32)
B2_EMB = 0                  # [10, 64]
B2_CB = B2_EMB + 64         # [10, 64]
B2_ID = B2_CB + 64          # [16, 16]
B2_E4 = B2_ID + 16          # [4, 20]  E4[b',(b g)] = delta(b,b')
B2_ME = B2_E4 + 20          # [10, 20] mask_even[n,(b g)] = delta(n,2g)
B2_MO = B2_ME + 20          # [10, 20] mask_odd[n,(b g)]  = delta(n,2g+1)
B2_B1 = B2_MO + 20          # [128, 1]
B2_B2 = B2_B1 + 1           # [64, 1]
B2_D = B2_B2 + 1            # 206
BBT_D = BB_D + 2 * B2_D     # total bf16 cols of the merged block


def fix_sync_waits(nc, cap=2):
    """This walrus build allows at most `cap` sem waits per instruction.
    Splice same-engine NoOps carrying the excess waits right before any
    over-subscribed instruction (waits happen earlier => same semantics)."""
    uid = [0]
    for f in nc.m.functions:
        for blk in f.blocks:
            insts = blk.instructions  # live list
            i = 0
            while i < len(insts):
                inst = insts[i]
                si = inst.sync_info
                waits = list(si.on_wait) if si and si.on_wait else []
                icap = 1
                if len(waits) <= icap:
                    i += 1
                    continue
                keep, excess = waits[-icap:], waits[:-icap]
                for k in range(0, len(excess), icap):
                    nop = mybir.InstNoOp(
                        name=f"{inst.name}-wsplit{uid[0]}", ins=[], outs=[]
                    )
                    uid[0] += 1
                    nop.engine = inst.engine
                    nop.sync_info = mybir.SyncInfo(
                        on_wait=excess[k : k + icap], on_update=[]
                    )
                    nc.register_instruction(nop, overwrite=True)
                    insts.insert(i, nop)
                    i += 1
                inst.sync_info = mybir.SyncInfo(
                    on_wait=keep,
                    on_update=list(si.on_update) if si and si.on_update else [],
                )
                i += 1


def build():
    nc = bass.Bass(num_swdge_queues=4)
    blkb = nc.dram_tensor("blkb", [128, BBT_D], BF16, kind="ExternalInput")
    cwp = nc.dram_tensor("cwp", [128, NG, FD], BF16, kind="ExternalInput")
    x = nc.dram_tensor("x", [BLOC, CIN, HW], BF16, kind="ExternalInput")
    out = nc.dram_tensor("out", [BLOC, COUT, OH, OW], F32, kind="ExternalOutput")

    with TileContext(nc) as tc:
        with (
            tc.tile_pool(name="consts", bufs=1) as consts,
            tc.tile_pool(name="work", bufs=2) as work,
            tc.tile_pool(name="stage", bufs=3) as stage,
            tc.tile_pool(name="ps", bufs=2, space="PSUM") as pspool,
            tc.tile_pool(name="pswfsel", bufs=1, space="PSUM") as pswfsel,
            tc.tile_pool(name="psconv", bufs=2, space="PSUM") as psconv,
        ):
            # ---------- DMA in (all on sync hw queue, critical first) ----------
            bbt = consts.tile([128, BBT_D], BF16, tag="bbt")
            nc.sync.dma_start(out=bbt[:], in_=blkb[:])
            b2t = bbt[:, BB_D : BB_D + 2 * B2_D].bitcast(F32)
            cwt = consts.tile([128, NG, FD], BF16, tag="cwt")
            nc.sync.dma_start(out=cwt[:, 0:3], in_=cwp[:, 0:3])
            nc.sync.dma_start(out=cwt[:, 3:NG], in_=cwp[:, 3:NG])

            xt = []
            for j in range(2):
                t = consts.tile([128, HW + 4], BF16, tag=f"xt{j}")
                nc.vector.memset(t[:, HW : HW + 4], 0.0)
                xt.append(t)
            for b in range(BLOC):
                j, half = divmod(b, 2)
                nc.sync.dma_start(
                    out=xt[j][64 * half : 64 * half + 64, 0:HW], in_=x[b]
                )

            ones64 = consts.tile([EDIM, 1], F32, tag="ones64")
            nc.vector.memset(ones64[:], 1.0)
            ones10_64 = consts.tile([NB, 64], F32, tag="ones10_64")
            nc.vector.memset(ones10_64[:], 1.0)
            ones4_128 = consts.tile([BLOC, 128], F32, tag="ones4_128")
            nc.vector.memset(ones4_128[:], 1.0)
            # first table-func activation triggers the 1.3us ACT_TABLE_LOAD;
            # issue a dummy now so it overlaps the input DMA wait
            actwarm = consts.tile([1, 1], F32, tag="actwarm")
            nc.scalar.activation(out=actwarm[:], in_=ones64[0:1], func=AF.Exp)

            # views into the packed blocks
            rvT = bbt[:, BB_RVT : BB_RVT + 16].rearrange("p (c b) -> p c b", c=4)
            w1sb = bbt[:, BB_W1 : BB_W1 + 512].rearrange("p (c m) -> p c m", c=4)
            w2sb = bbt[:, BB_W2 : BB_W2 + 64]
            identPair = bbt[:, BB_IP : BB_IP + 64]
            embsb = b2t[0:NB, B2_EMB : B2_EMB + 64]
            cbsb = b2t[0:NB, B2_CB : B2_CB + 64]
            ident = b2t[0:16, B2_ID : B2_ID + 16]
            e4sb = b2t[0:4, B2_E4 : B2_E4 + 20]
            maskE = b2t[0:NB, B2_ME : B2_ME + 20]
            maskO = b2t[0:NB, B2_MO : B2_MO + 20]
            b1sb = b2t[:, B2_B1 : B2_B1 + 1]
            b2sb = b2t[0:EDIM, B2_B2 : B2_B2 + 1]

            # ---------- embedding norms (independent of rv; runs early) -------
            esq = work.tile([NB, EDIM], F32, tag="esq")
            nc.vector.tensor_mul(esq[:], embsb, embsb)
            ensq = work.tile([NB, 1], F32, tag="ensq")
            nc.vector.tensor_reduce(ensq[:], esq[:], axis=AX.X, op=ALU.add)
            eln = work.tile([NB, 1], F32, tag="eln")
            nc.scalar.activation(out=eln[:], in_=ensq[:], func=AF.Ln)
            einv = work.tile([NB, 1], F32, tag="einv")
            nc.scalar.activation(out=einv[:], in_=eln[:], func=AF.Exp, scale=-0.5)
            embn = work.tile([NB, EDIM], F32, tag="embn")
            nc.vector.tensor_scalar_mul(out=embn[:], in0=embsb, scalar1=einv[:])
            embnT_ps = pspool.tile([EDIM, NB], F32, tag="embp", bufs=1)
            nc.tensor.transpose(embnT_ps[:], embn[:], ident[0:NB, 0:NB])
            embnT = work.tile([EDIM, NB], F32, tag="embnT")
            nc.scalar.copy(out=embnT[:], in_=embnT_ps[:])

            # ---------- routing MLP (bf16 matmuls; rv pre-transposed) ----------
            h1 = pspool.tile([HID, BLOC], F32, tag="small")
            for c in range(4):
                nc.tensor.matmul(
                    h1[:], w1sb[:, c, :], rvT[:, c, :], start=(c == 0), stop=(c == 3)
                )
            h1r = work.tile([HID, BLOC], BF16, tag="h1r")
            nc.scalar.activation(
                out=h1r[:], in_=h1[:], func=AF.Relu, bias=b1sb, scale=1.0
            )
            rps = pspool.tile([EDIM, BLOC], F32, tag="small")
            nc.tensor.matmul(rps[:], w2sb, h1r[:], start=True, stop=True)
            rsb = work.tile([EDIM, BLOC], F32, tag="rsb")
            nc.scalar.activation(
                out=rsb[:], in_=rps[:], func=AF.Identity, bias=b2sb, scale=1.0
            )

            # ---------- r norms (rsq straight off the PSUM result) ----------
            rsq = work.tile([EDIM, BLOC], F32, tag="rsq")
            nc.scalar.activation(
                out=rsq[:], in_=rps[:], func=AF.Square, bias=b2sb, scale=1.0
            )
            nsq = pspool.tile([BLOC, 1], F32, tag="small")
            nc.tensor.matmul(nsq[:], rsq[:], ones64[:], start=True, stop=True)
            rln = work.tile([BLOC, 1], F32, tag="rln")
            nc.scalar.activation(out=rln[:], in_=nsq[:], func=AF.Ln)
            rinv = work.tile([BLOC, 1], F32, tag="rinv")
            nc.scalar.activation(out=rinv[:], in_=rln[:], func=AF.Exp, scale=-0.5)

            # ---------- similarity + unnormalized softmax numerators ----------
            # cosine sims are in [-1,1]: exp() cannot overflow, so skip the
            # max-subtraction; 1/sum is applied at PSUM evacuation time.
            simps = pspool.tile([BLOC, NB], F32, tag="small")
            nc.tensor.matmul(simps[:], rsb[:], embnT[:], start=True, stop=True)
            ex = work.tile([BLOC, NB], F32, tag="ex")
            s = work.tile([BLOC, 1], F32, tag="s")
            nc.scalar.activation(
                out=ex[:], in_=simps[:], func=AF.Exp, scale=rinv[:],
                accum_out=s[:],
            )

            # ---------- per-(sample, expert-pair) scale table ----------
            # wfsel[p, (b g)] = ex[b, 2g] for p < 64, ex[b, 2g+1] for p >= 64
            wfx_ps = pspool.tile([NB, 20], F32, tag="small")
            nc.tensor.matmul(wfx_ps[:], ex[:], e4sb, start=True, stop=True)
            exT_ps = pspool.tile([NB, BLOC], F32, tag="embp", bufs=1)
            nc.tensor.transpose(exT_ps[:], ex[:], ident[0:BLOC, 0:BLOC])
            exT = work.tile([NB, BLOC], F32, tag="exT")
            nc.scalar.copy(out=exT[:], in_=exT_ps[:])
            rhsE = work.tile([NB, 20], F32, tag="rhsE")
            nc.vector.tensor_mul(rhsE[:], wfx_ps[:], maskE)
            rhsO = work.tile([NB, 20], F32, tag="rhsO")
            nc.vector.tensor_mul(rhsO[:], wfx_ps[:], maskO)
            wfsel_ps = pswfsel.tile([128, 20], F32, tag="wfselp")
            nc.tensor.matmul(
                wfsel_ps[0:64], ones10_64[:], rhsE[:], start=True, stop=True,
                tile_position=(0, 0), skip_group_check=True,
            )
            nc.tensor.matmul(
                wfsel_ps[64:128], ones10_64[:], rhsO[:], start=True, stop=True,
                tile_position=(0, 64), skip_group_check=True,
            )
            wfsel = work.tile([128, 20], F32, tag="wfsel")
            nc.scalar.copy(out=wfsel[:], in_=wfsel_ps[:])

            # ---------- normalizer broadcast + unnormalized bias ----------
            sinv = work.tile([BLOC, 1], F32, tag="sinv")
            nc.vector.reciprocal(sinv[:], s[:])
            diag4 = work.tile([BLOC, BLOC], F32, tag="diag4")
            nc.vector.tensor_scalar_mul(
                out=diag4[:], in0=ident[0:BLOC, 0:BLOC], scalar1=sinv[:]
            )
            sinvb_ps = pspool.tile([128, BLOC], F32, tag="embp", bufs=1)
            nc.tensor.matmul(
                sinvb_ps[:], ones4_128[:], diag4[:], start=True, stop=True
            )
            sinvb = work.tile([128, BLOC], F32, tag="sinvb")
            nc.scalar.copy(out=sinvb[:], in_=sinvb_ps[:])
            # beffU = cb^T ex (unnormalized); evacuation computes (pv+bU)/s
            beff_ps = pspool.tile([128, BLOC], F32, tag="embp", bufs=1)
            nc.tensor.matmul(
                beff_ps[0:64], cbsb, exT[:], start=True, stop=True,
                tile_position=(0, 0),
            )
            nc.tensor.matmul(
                beff_ps[64:128], cbsb, exT[:], start=True, stop=True,
                tile_position=(0, 64), skip_group_check=True,
            )
            beffU = work.tile([128, BLOC], F32, tag="beffU")
            nc.scalar.copy(out=beffU[:], in_=beff_ps[:])

            # scaled-identity stationaries lhsw[b][g]=[ex[b,2g]*I; ex[b,2g+1]*I]
            # vector builds even samples off the PSUM table, scalar the odd
            # samples off the SBUF copy
            lhsw = [[None] * NG for _ in range(BLOC)]
            for b in range(BLOC):
                for g in range(NG):
                    lhsw[b][g] = work.tile(
                        [128, 64], BF16, tag=f"lh{b}_{g}", name=f"lh{b}_{g}"
                    )
            for b in (0, 2):
                for g in range(NG):
                    col = 5 * b + g
                    nc.vector.tensor_scalar_mul(
                        out=lhsw[b][g][:], in0=identPair,
                        scalar1=wfsel_ps[:, col : col + 1],
                    )
            for b in (1, 3):
                for g in range(NG):
                    col = 5 * b + g
                    nc.scalar.activation(
                        out=lhsw[b][g][:], in_=identPair, func=AF.Identity,
                        scale=wfsel[:, col : col + 1],
                    )

            # ---------- w_eff via PSUM-accumulated matmuls ----------
            # weff[0:64]  = weights for sample 2j   (PE row tile 0)
            # weff[64:128] = weights for sample 2j+1 (PE row tile 64)
            weffs = []
            for j in range(2):
                wpsA = psconv.tile([128, NFREE], F32, tag="psA")
                wpsB = psconv.tile([128, NFREE], F32, tag="psB")
                for g in range(NG):
                    st_, sp = (g == 0), (g == NG - 1)
                    for half, b in ((0, 2 * j), (1, 2 * j + 1)):
                        lo, hi = 64 * half, 64 * half + 64
                        nc.tensor.matmul(
                            wpsA[lo:hi, 0:FDH], lhsw[b][g], cwt[:, g, 0:FDH],
                            start=st_, stop=sp, tile_position=(0, 64 * half),
                            skip_group_check=True,
                        )
                        nc.tensor.matmul(
                            wpsB[lo:hi, 0:FDH], lhsw[b][g], cwt[:, g, FDH:FD],
                            start=st_, stop=sp, tile_position=(0, 64 * half),
                            skip_group_check=True,
                        )
                weff = work.tile(
                    [128, NTAP, COUT], BF16, tag=f"weff{j}", name=f"weff{j}"
                )
                wv = weff[:].rearrange("p t c -> p (t c)")
                nc.vector.tensor_copy(out=wv[:, 0:FDH], in_=wpsA[:, 0:FDH])
                nc.vector.tensor_copy(out=wv[:, FDH:FD], in_=wpsB[:, 0:FDH])
                weffs.append(weff)

            # ---------- conv: quad-tiled 9-tap shifted matmuls ----------
            for j in range(2):
                weff = weffs[j]
                for g in range(NGRP):
                    h_e = 2 * g * GR       # even chunk start row
                    h_o = h_e + GR         # odd chunk start row
                    psA = psconv.tile([128, NFREE], F32, tag="psA")
                    psB = psconv.tile([128, NFREE], F32, tag="psB")
                    for t in range(NTAP):
                        oe = h_e * W + TAP_OFF[t]
                        oo = h_o * W + TAP_OFF[t]
                        st_, sp = (t == 0), (t == NTAP - 1)
                        nc.tensor.matmul(
                            psA[0:64], weff[0:64, t, :], xt[j][0:64, oe : oe + NFREE],
                            start=st_, stop=sp, tile_position=(0, 0),
                            skip_group_check=True,
                        )
                        nc.tensor.matmul(
                            psA[64:128], weff[0:64, t, :], xt[j][0:64, oo : oo + NFREE],
                            start=st_, stop=sp, tile_position=(0, 64),
                            skip_group_check=True,
                        )
                        nc.tensor.matmul(
                            psB[0:64], weff[64:128, t, :], xt[j][64:128, oe : oe + NFREE],
                            start=st_, stop=sp, tile_position=(64, 0),
                            skip_group_check=True,
                        )
                        nc.tensor.matmul(
                            psB[64:128], weff[64:128, t, :], xt[j][64:128, oo : oo + NFREE],
                            start=st_, stop=sp, tile_position=(64, 64),
                            skip_group_check=True,
                        )
                    # evacuate on vector ((pv + beffU) * 1/s); store each
                    # row-parity immediately (sync + scalar hw queues)
                    for half, ps in ((0, psA), (1, psB)):
                        b = 2 * j + half
                        st = stage.tile(
                            [128, GR, OW], F32,
                            tag=f"stg{half}", name=f"stg{half}",
                        )
                        pv = ps[:].rearrange("p (r w) -> p r w", w=W)[:, :, 0:OW]
                        nc.vector.tensor_scalar(
                            out=st[:], in0=pv,
                            scalar1=beffU[:, b : b + 1],
                            scalar2=sinvb[:, b : b + 1],
                            op0=ALU.add, op1=ALU.mult,
                        )
                        dv = out[b].rearrange(
                            "c (G g2 r) w -> g2 c G r w", G=NGRP, g2=2
                        )
                        nc.sync.dma_start(out=dv[0, :, g], in_=st[0:64])
                        nc.scalar.dma_start(out=dv[1, :, g], in_=st[64:128])

    fix_sync_waits(nc)
    return nc


_NC = None


def _get_nc():
    global _NC
    if _NC is None:
        _NC = build()
    return _NC


def make_in_maps(inputs):
    bf16 = ml_dtypes.bfloat16
    x = np.asarray(inputs["x"], dtype=np.float32).reshape(32, CIN, HW)
    rvec = np.asarray(inputs["routing_vector"], dtype=np.float32)
    W1 = np.asarray(inputs["W1"], dtype=np.float32)
    b1 = np.asarray(inputs["b1"], dtype=np.float32)
    W2 = np.asarray(inputs["W2"], dtype=np.float32)
    b2 = np.asarray(inputs["b2"], dtype=np.float32)
    emb = np.asarray(inputs["emb"], dtype=np.float32)
    conv_w = np.asarray(inputs["conv_w"], dtype=np.float32)
    conv_b = np.asarray(inputs["conv_b"], dtype=np.float32)

    # conv_w[n, co, ci, ky, kx] -> cwp[(n%2)*64+ci, n//2, (ky kx)*co], bf16
    cwpt = conv_w.transpose(2, 0, 3, 4, 1).reshape(CIN, NB, FD)   # [ci, n, tc]
    cwpa = np.zeros((128, NG, FD), np.float32)
    for g in range(NG):
        cwpa[0:64, g] = cwpt[:, 2 * g]
        cwpa[64:128, g] = cwpt[:, 2 * g + 1]
    cwpa = np.ascontiguousarray(cwpa).astype(bf16)

    # blkb (bf16): per-core rvT + routing weights + stacked identity pair
    w1blk = W1.reshape(4, 128, HID).transpose(1, 0, 2).reshape(128, 512)
    blkb_shared = np.zeros((128, BB_D), np.float32)
    blkb_shared[:, BB_W1 : BB_W1 + 512] = w1blk
    blkb_shared[:, BB_W2 : BB_W2 + 64] = W2
    eye64 = np.eye(64, dtype=np.float32)
    blkb_shared[0:64, BB_IP : BB_IP + 64] = eye64
    blkb_shared[64:128, BB_IP : BB_IP + 64] = eye64

    # blk2 (fp32): emb, conv bias, identity, wfsel helper constants, biases
    blk2a = np.zeros((128, B2_D), np.float32)
    blk2a[0:NB, B2_EMB : B2_EMB + 64] = emb
    blk2a[0:NB, B2_CB : B2_CB + 64] = conv_b
    blk2a[0:16, B2_ID : B2_ID + 16] = np.eye(16, dtype=np.float32)
    for b in range(BLOC):
        for g in range(NG):
            blk2a[b, B2_E4 + 5 * b + g] = 1.0
            blk2a[2 * g, B2_ME + 5 * b + g] = 1.0
            blk2a[2 * g + 1, B2_MO + 5 * b + g] = 1.0
    blk2a[:, B2_B1] = b1
    blk2a[0:EDIM, B2_B2] = b2

    xb = x.astype(bf16)
    in_maps = []
    for c in range(NCORES):
        blkba = blkb_shared.copy()
        rvc = rvec[BLOC * c : BLOC * (c + 1)]          # [4, 512]
        # rvT[p, c, b] = rv[b, 128c + p]
        rvt = rvc.T.reshape(4, 128, BLOC).transpose(1, 0, 2).reshape(128, 16)
        blkba[:, BB_RVT : BB_RVT + 16] = rvt
        merged = np.zeros((128, 2 * BBT_D), np.uint8)
        merged[:, : 2 * BB_D] = (
            blkba.astype(bf16).view(np.uint8).reshape(128, 2 * BB_D)
        )
        merged[:, 2 * BB_D :] = blk2a.view(np.uint8).reshape(128, 4 * B2_D)
        in_maps.append(
            {
                "blkb": np.ascontiguousarray(merged).view(bf16),
                "cwp": cwpa,
                "x": np.ascontiguousarray(xb[BLOC * c : BLOC * (c + 1)]),
            }
        )
    return in_maps


def kernel(**inputs):
    from concourse.bass_utils import run_bass_kernel_spmd

    nc = _get_nc()
    in_maps = make_in_maps(inputs)
    res = run_bass_kernel_spmd(nc, in_maps, core_ids=list(range(NCORES)))
    return np.concatenate([r["out"] for r in res.results], axis=0)
